# revision 15
# baseline (speedup 1.0000x reference)
"""Low-rank (random-feature) attention kernel for Trainium2, 8 NeuronCores — v3.

Sharding: flatten hidden_states to [B*S, H] = [32768, 768] rows; core c owns
4096 contiguous rows (= batch c//2, sequence half c%2).  The per-(batch, head)
kv summary is reduced with two pipelined pairwise AllReduces (~200 KB each).

v3 structural changes vs v2 (425 us):
- Phase 1 row-tile loop reordered (all kp matmuls, then emit_kv(rt-1), then
  all v matmuls) with kp_ps double-buffered and the kv accumulators single-
  buffered: PSUM = 4+2+2 = 8 banks, and the per-tile exp/max drain now has a
  full GEMM of slack -> removes the 1.7 us PE stall per row tile (~55 us).
- bkp is folded into the q-side exp bias on the host (qp' = exp(x Wqp + bqp
  + bkp) scales numerator and denominator identically), and bv rides bo as
  bo' = bo + bv @ Wo (exact up to an O(eps/n) ~ 1e-6 term).  The whole
  post-collective kv fixup chain (ebkp/bvbc scalar_tensor_tensor ops) is
  gone; after the AllReduce only dtype-converting block-diag copies remain,
  so the normalizer matmuls no longer stall ~19 us on DVE.
- bd_kv / ks_bd memsets hoisted to the DMA load phase.
- xT row-tile loads split across the sync and vector DGE queues (halves the
  time to first real matmul).
- bf16 everywhere off-chip (including the output, cast back on host).
"""

import sys

sys.path.insert(0, "/opt/trn_rl_repo")

import contextlib

import ml_dtypes
import numpy as np

import concourse.bass as bass
import concourse.tile as tile
from concourse import mybir
from concourse.bass_utils import run_bass_kernel_spmd

BF16 = mybir.dt.bfloat16
F32 = mybir.dt.float32
FP8 = mybir.dt.float8e4
AF = mybir.ActivationFunctionType
ALU = mybir.AluOpType
AX = mybir.AxisListType

B, S, H = 4, 8192, 768
NH, HD, M = 12, 64, 64
EPS = 1e-6
NCORES = 8
R = (B * S) // NCORES          # rows per core = 4096
NPAIR = NH // 2                # head pairs = 6
KT = H // 128                  # hidden k-tiles = 6
RT = 128                       # row tile
NRT = R // RT                  # 32 row tiles
CHUNK = 512
NCH = R // CHUNK               # 8 chunks
WARM_MM = 24                   # PE warm-up matmuls during the load phase


def _split_multi_waits(nc):
    """This container's walrus only accepts one semaphore wait per
    instruction; hoist extra waits onto same-engine NoOps placed before."""
    c = 0
    for f in nc.m.functions:
        for bb in f.blocks:
            new_insts = []
            for ins in bb.instructions:
                si = ins.sync_info
                if si is not None and si.on_wait and len(si.on_wait) > 1:
                    waits = list(si.on_wait)
                    for w in waits[:-1]:
                        c += 1
                        new_insts.append(mybir.InstNoOp(
                            name=f"I-waitsplit-{c}", engine=ins.engine,
                            sync_info=mybir.SyncInfo(on_wait=[w], on_update=[])))
                    ins.sync_info = mybir.SyncInfo(
                        on_wait=[waits[-1]], on_update=list(si.on_update))
                new_insts.append(ins)
            bb.instructions[:] = new_insts
    return c


def _bc_free(ap, n):
    """Broadcast an AP along a new innermost (stride-0) free axis of size n."""
    return bass.AP(tensor=ap.tensor, offset=ap.offset, ap=list(ap.ap) + [[0, n]])


def _build(nc):
    xt = nc.dram_tensor("xt", [H, R], BF16, kind="ExternalInput")
    wkp = nc.dram_tensor("wkp", [H, H], BF16, kind="ExternalInput")
    wv = nc.dram_tensor("wv", [H, H], BF16, kind="ExternalInput")
    wqp = nc.dram_tensor("wqp", [H, H], FP8, kind="ExternalInput")
    wo = nc.dram_tensor("wo", [H, H], BF16, kind="ExternalInput")
    bqpc_d = nc.dram_tensor("bqpc", [128, KT], F32, kind="ExternalInput")
    bobc_d = nc.dram_tensor("bobc", [128, H], F32, kind="ExternalInput")
    sel12_d = nc.dram_tensor("sel12", [NH, NPAIR, 128], BF16, kind="ExternalInput")
    out = nc.dram_tensor("out", [R, H], BF16, kind="ExternalOutput")

    with tile.TileContext(nc) as tc, contextlib.ExitStack() as ctx:
        persist = ctx.enter_context(tc.tile_pool(name="persist", bufs=1))
        dram = ctx.enter_context(tc.tile_pool(name="dram", bufs=1, space="DRAM"))

        # ---- PE warm-up fodder: available immediately (no DMA dependency) ----
        junk = persist.tile([128, 512], BF16, tag="junk", name="junk")
        nc.vector.memset(junk[:], 1.0)

        # ---- constants (gpsimd SWDGE queue, small) ----
        bqpc = persist.tile([128, KT], F32, tag="bqpc", name="bqpc")
        nc.gpsimd.dma_start(out=bqpc[:], in_=bqpc_d.ap())
        bobc = persist.tile([128, H], F32, tag="bobc", name="bobc")
        nc.gpsimd.dma_start(out=bobc[:], in_=bobc_d.ap())
        sel12 = persist.tile([NH, NPAIR, 128], BF16, tag="sel12", name="sel12")
        nc.gpsimd.dma_start(out=sel12[:], in_=sel12_d.ap())

        # ---- weights + xT interleaved across the sync and scalar HWDGE rings.
        # Per-ring order is first-needed-first: wkp halves, then x chunk 0
        # (first 4 row tiles), then wv halves (one ~3 us stall at v(0) beats
        # pushing kp(0) back), then the rest ahead of phase-1 consumption.
        wkp_sb = persist.tile([128, KT, H], BF16, tag="wkp", name="wkp_sb")
        wv_sb = persist.tile([128, KT, H], BF16, tag="wv", name="wv_sb")
        wqp_sb = persist.tile([128, KT, H], FP8, tag="wqp", name="wqp_sb")
        wo_sb = persist.tile([128, KT, H], BF16, tag="wo", name="wo_sb")
        xt_sb = persist.tile([128, KT, R], BF16, tag="xt", name="xt_sb")

        def load_w(wsb, wdram, half, eng):
            ks = slice(0, 3) if half == 0 else slice(3, KT)
            eng.dma_start(out=wsb[:, ks, :],
                          in_=wdram.ap().rearrange("(k p) n -> p k n", p=128)[:, ks, :])

        # kp(0) is gated by wkp + the first x strip, so they get a full ring
        # each (2KB+ lines; the 512-col strips of v4 ran at half DMA rate and
        # starved tiles 4-7 for ~17 us).  wv lands per-kt just ahead of v(0);
        # later strips and wqp/wo trail well ahead of their consumers.
        def load_x(c0, c1, eng):
            for kt in range(KT):
                eng.dma_start(out=xt_sb[:, kt, c0:c1],
                              in_=xt[kt * 128:(kt + 1) * 128, c0:c1])

        def load_wv(kt, eng):
            eng.dma_start(out=wv_sb[:, kt, :],
                          in_=wv.ap().rearrange("(k p) n -> p k n", p=128)[:, kt, :])

        load_w(wkp_sb, wkp, 0, nc.scalar)   # ring B: wkp halves back-to-back
        load_w(wkp_sb, wkp, 1, nc.scalar)
        load_x(0, 1024, nc.sync)            # ring A: first 8 row tiles
        for kt in range(3):
            load_wv(kt, nc.sync)
        for kt in range(3, KT):
            load_wv(kt, nc.scalar)
        load_x(1024, 2560, nc.scalar)
        load_x(2560, 4096, nc.sync)
        load_w(wqp_sb, wqp, 0, nc.sync); load_w(wqp_sb, wqp, 1, nc.scalar)
        load_w(wo_sb, wo, 0, nc.sync); load_w(wo_sb, wo, 1, nc.scalar)

        # ---- persistent result tiles ----
        qpT = [persist.tile([128, R], BF16, tag=f"qpT{p}", name=f"qpT{p}")
               for p in range(NPAIR)]
        kv_acc = [persist.tile([128, NPAIR, 130], F32, tag=f"kvacc{h}",
                               name=f"kv_acc{h}") for h in range(2)]
        kv_fix = persist.tile([128, NPAIR, 130], F32, tag="kvfix", name="kv_fix")
        bd_kv = persist.tile([128, NPAIR, 128], BF16, tag="bdkv", name="bd_kv")
        ks_bd = persist.tile([128, NPAIR, NH], BF16, tag="ksbd", name="ks_bd")
        r_cat = persist.tile([NH, R], BF16, tag="rcat", name="r_cat")
        # block-diag scaffolding zeroed while DVE is otherwise idle
        nc.vector.memset(bd_kv[:], 0.0)
        nc.vector.memset(ks_bd[:], 0.0)

        cc_in = [dram.tile([128, NPAIR * 130], F32, name=f"cc_in{h}") for h in range(2)]
        cc_out = [dram.tile([128, NPAIR * 130], F32, name=f"cc_out{h}") for h in range(2)]

        # ================= Phase 1: k/v pass + kv accumulation =================
        with contextlib.ExitStack() as kctx:
            kvsb = kctx.enter_context(tc.tile_pool(name="kvsb", bufs=2))
            kvps = kctx.enter_context(tc.tile_pool(name="kvps", bufs=1, space="PSUM"))

            # warm-up matmuls share the kp_ps PSUM slot (bufs=2)
            warm_ps = kvps.tile([128, H], F32, tag="kp_ps", bufs=2, name="warm_ps")
            for i in range(WARM_MM):
                nc.tensor.matmul(warm_ps[:, 0:512], junk[:, 0:128], junk[:, :],
                                 start=True, stop=True)

            kv_ps = [None, None]   # [kv_a, kv_b] of current half
            kp_sbs = {}
            v_sbs = {}

            def emit_kv(rt):
                hh, rl = rt // (NRT // 2), rt % (NRT // 2)
                if rl == 0:
                    kv_ps[0] = kvps.tile([128, 3, 130], F32, tag="kv_a",
                                         bufs=1, name=f"kv_a{hh}")
                    kv_ps[1] = kvps.tile([128, 3, 130], F32, tag="kv_b",
                                         bufs=1, name=f"kv_b{hh}")
                kp_sb, v_sb = kp_sbs.pop(rt), v_sbs.pop(rt)
                for p in range(NPAIR):
                    # start=True clears has_written for the WHOLE bank, so only
                    # the first region of each bank may issue it; the other two
                    # pairs overwrite-on-clear at rl==0 via has_written=0.
                    nc.tensor.matmul(
                        kv_ps[p // 3][:, p % 3, :],
                        kp_sb[:, p * 128:(p + 1) * 128],
                        v_sb[:, 2 * p:2 * p + 2, :],
                        start=(rl == 0 and p % 3 == 0),
                        stop=(rl == NRT // 2 - 1),
                        skip_group_check=True)

            def drain_kv(hh):
                nc.vector.tensor_copy(out=kv_acc[hh][:, 0:3, :], in_=kv_ps[0][:])
                nc.vector.tensor_copy(out=kv_acc[hh][:, 3:6, :], in_=kv_ps[1][:])
                nc.sync.dma_start(out=cc_in[hh][:],
                                  in_=kv_acc[hh].rearrange("p a b -> p (a b)"))
                nc.gpsimd.collective_compute(
                    "AllReduce", ALU.add,
                    replica_groups=[[0, 1], [2, 3], [4, 5], [6, 7]],
                    ins=[cc_in[hh].opt()], outs=[cc_out[hh].opt()])

            for rt in range(NRT):
                rs = slice(rt * RT, (rt + 1) * RT)
                kp_ps = kvps.tile([128, H], F32, tag="kp_ps", bufs=2,
                                  name=f"kp_ps{rt}")
                v_ps = kvps.tile([128, H], F32, tag="v_ps", name=f"v_ps{rt}")
                for kt in range(KT):
                    xblk = xt_sb[:, kt, rs]
                    st, sp = (kt == 0), (kt == KT - 1)
                    nc.tensor.matmul(kp_ps[:, 0:512], xblk, wkp_sb[:, kt, 0:512],
                                     start=st, stop=sp)
                    nc.tensor.matmul(kp_ps[:, 512:768], xblk, wkp_sb[:, kt, 512:768],
                                     start=st, stop=sp)
                # kv outer products for the previous row tile sit between the
                # kp and v GEMMs of this one, so the previous tile's exp/scale
                # chain and this tile's kp drain both have a GEMM of slack
                if rt > 0:
                    emit_kv(rt - 1)
                    if rt == NRT // 2:
                        drain_kv(0)
                for kt in range(KT):
                    xblk = xt_sb[:, kt, rs]
                    st, sp = (kt == 0), (kt == KT - 1)
                    nc.tensor.matmul(v_ps[:, 0:512], xblk, wv_sb[:, kt, 0:512],
                                     start=st, stop=sp)
                    nc.tensor.matmul(v_ps[:, 512:768], xblk, wv_sb[:, kt, 512:768],
                                     start=st, stop=sp)
                # postprocess this row tile
                nmx = kvsb.tile([128, NH], F32, tag="nmx", name=f"nmx{rt}")
                nc.vector.tensor_reduce(
                    out=nmx[:], in_=kp_ps.rearrange("p (h m) -> p h m", m=M),
                    op=ALU.max, axis=AX.X, negate=True)
                emax = kvsb.tile([128, NH], F32, tag="emax", name=f"emax{rt}")
                nc.scalar.activation(out=emax[:], in_=nmx[:], func=AF.Exp)
                kp_sb = kvsb.tile([128, H], BF16, tag="kp_sb", name=f"kp_sb{rt}")
                nc.scalar.activation(out=kp_sb[:], in_=kp_ps[:], func=AF.Exp)
                v_sb = kvsb.tile([128, NH, 65], BF16, tag="v_sb", name=f"v_sb{rt}")
                nc.vector.tensor_tensor(
                    out=v_sb[:, :, 0:64],
                    in0=v_ps.rearrange("p (h d) -> p h d", d=HD),
                    in1=_bc_free(emax[:], HD), op=ALU.mult)
                nc.vector.tensor_copy(out=v_sb[:, :, 64:65], in_=emax[:])
                kp_sbs[rt], v_sbs[rt] = kp_sb, v_sb
            emit_kv(NRT - 1)
            drain_kv(1)

        # ---- collective readback + block-diag assembly (copies only) ----
        kv_r = [persist.tile([128, NPAIR, 130], F32, tag=f"kvr{h}", name=f"kv_r{h}")
                for h in range(2)]
        for hh in range(2):
            nc.sync.dma_start(out=kv_r[hh][:],
                              in_=cc_out[hh].rearrange("p (a b) -> p a b", b=130))
        nc.vector.tensor_add(out=kv_fix[:], in0=kv_r[0][:], in1=kv_r[1][:])
        for p in range(NPAIR):
            nc.vector.tensor_copy(out=bd_kv[0:64, p, 0:64], in_=kv_fix[0:64, p, 0:64])
            nc.vector.tensor_copy(out=bd_kv[64:128, p, 64:128],
                                  in_=kv_fix[64:128, p, 65:129])
            nc.vector.tensor_copy(out=ks_bd[0:64, p, 2 * p:2 * p + 1],
                                  in_=kv_fix[0:64, p, 64:65])
            nc.vector.tensor_copy(out=ks_bd[64:128, p, 2 * p + 1:2 * p + 2],
                                  in_=kv_fix[64:128, p, 129:130])

        # ================= Phase 2: q pass, then normalizer =================
        with contextlib.ExitStack() as qctx:
            qsb = qctx.enter_context(tc.tile_pool(name="qsb", bufs=2))
            qps = qctx.enter_context(tc.tile_pool(name="qps", bufs=1, space="PSUM"))

            def norm_ch(ch):
                # 1/n computed as exp(-ln n) on the scalar engine: n is O(1e3)
                # and positive so eps and the Ln LUT error are negligible,
                # while the DVE reciprocal (3.3 us per chunk, serial) was
                # gating phase 3's PSUM bank reuse by ~12 us.
                cs = slice(ch * CHUNK, (ch + 1) * CHUNK)
                n_ps = qps.tile([NH, CHUNK], F32, tag="np", bufs=3, name=f"n_ps{ch}")
                for p in range(NPAIR):
                    nc.tensor.matmul(n_ps[:], ks_bd[:, p, :], qpT[p][:, cs],
                                     start=(p == 0), stop=(p == NPAIR - 1))
                ln_sb = qsb.tile([NH, CHUNK], F32, tag="lnn", bufs=2,
                                 name=f"ln_n{ch}")
                nc.scalar.activation(out=ln_sb[:], in_=n_ps[:], func=AF.Ln)
                nc.scalar.activation(out=r_cat[:, cs], in_=ln_sb[:], func=AF.Exp,
                                     scale=-1.0)

            for ch in range(NCH):
                cs = slice(ch * CHUNK, (ch + 1) * CHUNK)
                # q-side runs in fp8 DoubleRow (x16 and wqp x256 land both
                # operands in e4m3 range; the 1/4096 rides the exp's scale).
                # The k-side must stay bf16 (fp8 kp alone costs 2e-2 rel err),
                # the q-side alone costs ~9e-3 against the 2e-2 budget.
                xt8 = qsb.tile([128, KT, CHUNK], FP8, tag="x8", bufs=2,
                               name=f"xt8_{ch}")
                nc.vector.tensor_copy(out=xt8[:], in_=xt_sb[:, :, cs])
                for ct in range(KT):
                    qp_ps = qps.tile([128, CHUNK], F32, tag="qp", bufs=3,
                                     name=f"qp_ps{ch}_{ct}")
                    for kt in range(0, KT, 2):
                        nc.tensor.matmul(
                            qp_ps[:],
                            wqp_sb[:, kt:kt + 2, ct * 128:(ct + 1) * 128],
                            xt8[:, kt:kt + 2, :],
                            start=(kt == 0), stop=(kt == KT - 2),
                            perf_mode=mybir.MatmulPerfMode.DoubleRow)
                    nc.scalar.activation(out=qpT[ct][:, cs], in_=qp_ps[:],
                                         func=AF.Exp, scale=1.0 / 4096.0,
                                         bias=bqpc[:, ct:ct + 1])
                # normalizer chunks ride along once ks_bd (collective) is
                # ready, paced across the remaining qp chunks
                if ch >= 4:
                    norm_ch(ch - 4)
            for ch in range(NCH - 4, NCH):
                norm_ch(ch)

        # ================= Phase 3: ctx + output projection =================
        with contextlib.ExitStack() as cctx:
            csb = cctx.enter_context(tc.tile_pool(name="csb", bufs=2))
            cps = cctx.enter_context(tc.tile_pool(name="cps", bufs=1, space="PSUM"))
            for ch in range(NCH):
                cs = slice(ch * CHUNK, (ch + 1) * CHUNK)
                ctx_ch = csb.tile([128, NPAIR, CHUNK], BF16, tag="ctx",
                                  name=f"ctx{ch}")
                for p in range(NPAIR):
                    rb_ps = cps.tile([128, CHUNK], F32, tag="rb", bufs=2,
                                     name=f"rb_ps{ch}_{p}")
                    nc.tensor.matmul(rb_ps[:], sel12[:, p, :], r_cat[:, cs],
                                     start=True, stop=True)
                    rb_sb = csb.tile([128, CHUNK], F32, tag="rbsb", bufs=2,
                                     name=f"rb_sb{ch}_{p}")
                    nc.scalar.activation(out=rb_sb[:], in_=rb_ps[:], func=AF.Copy)
                    a_ps = cps.tile([128, CHUNK], F32, tag="a", bufs=2,
                                    name=f"a_ps{ch}_{p}")
                    nc.tensor.matmul(a_ps[:], bd_kv[:, p, :], qpT[p][:, cs],
                                     start=True, stop=True)
                    nc.vector.tensor_tensor(out=ctx_ch[:, p, :], in0=a_ps[:],
                                            in1=rb_sb[:], op=ALU.mult)
                for r4 in range(CHUNK // RT):
                    rt = ch * (CHUNK // RT) + r4
                    rs = slice(r4 * RT, (r4 + 1) * RT)
                    o_ps = cps.tile([128, H], F32, tag="o", bufs=2,
                                    name=f"o_ps{rt}")
                    for p in range(NPAIR):
                        st, sp = (p == 0), (p == NPAIR - 1)
                        nc.tensor.matmul(o_ps[:, 0:512], ctx_ch[:, p, rs],
                                         wo_sb[:, p, 0:512], start=st, stop=sp)
                        nc.tensor.matmul(o_ps[:, 512:768], ctx_ch[:, p, rs],
                                         wo_sb[:, p, 512:768], start=st, stop=sp)
                    o_sb = csb.tile([128, H], BF16, tag="osb", bufs=3,
                                    name=f"o_sb{rt}")
                    nc.vector.tensor_tensor(out=o_sb[:], in0=o_ps[:], in1=bobc[:],
                                            op=ALU.add)
                    nc.sync.dma_start(out=out[rt * RT:(rt + 1) * RT, :], in_=o_sb[:])

    _split_multi_waits(nc)
    return nc


_CACHE = {}
TRACE = False          # set by test harness to capture an NTFF profile
LAST_EXEC_NS = None    # filled on a TRACE run


def _get_nc():
    if "nc" not in _CACHE:
        nc = bass.Bass("TRN2", target_bir_lowering=False, debug=False,
                       num_devices=NCORES)
        _CACHE["nc"] = _build(nc)
    return _CACHE["nc"]


def kernel(hidden_states, Wq, bq, Wk, bk, Wv, bv, Wo, bo, projection_matrix):
    nc = _get_nc()
    BFD = ml_dtypes.bfloat16
    xf = np.asarray(hidden_states, dtype=np.float32).reshape(B * S, H)
    xf = (xf * np.float32(16.0)).astype(BFD)
    pm = np.asarray(projection_matrix, dtype=np.float32)
    wq_f = np.asarray(Wq, dtype=np.float32)
    wk_f = np.asarray(Wk, dtype=np.float32)
    wo_f = np.asarray(Wo, dtype=np.float32)
    bq_f = np.asarray(bq, dtype=np.float32)
    bk_f = np.asarray(bk, dtype=np.float32)
    bv_f = np.asarray(bv, dtype=np.float32)
    bo_f = np.asarray(bo, dtype=np.float32)
    # fold the feature projection into the q and k weights (exact in fp32)
    wqp = np.zeros((H, H), np.float32)
    wkp = np.zeros((H, H), np.float32)
    bqp = np.zeros((H,), np.float32)
    bkp = np.zeros((H,), np.float32)
    for h in range(NH):
        cols = slice(h * HD, (h + 1) * HD)
        wqp[:, cols] = wq_f[:, cols] @ pm[h]
        wkp[:, cols] = wk_f[:, cols] @ pm[h]
        bqp[cols] = bq_f[cols] @ pm[h]
        bkp[cols] = bk_f[cols] @ pm[h]
    # k-side projected bias rides the q-side exp (it scales the ctx numerator
    # and denominator identically); bv rides bo through Wo.
    bqp = bqp + bkp
    bo_f = bo_f + bv_f @ wo_f
    bqpc = np.ascontiguousarray(bqp.reshape(KT, 128).T)            # [128, KT]
    bobc = np.ascontiguousarray(np.broadcast_to(bo_f, (128, H)).copy())
    sel12 = np.zeros((NH, NPAIR, 128), np.float32)
    for p in range(NPAIR):
        sel12[2 * p, p, 0:64] = 1.0
        sel12[2 * p + 1, p, 64:128] = 1.0
    # power-of-2 rescales (exact in bf16): x*16 lifts the fp8 copy of x out
    # of e4m3 subnormals, wkp/wv absorb the 1/16, wqp*256 centers the fp8
    # weights; the q GEMM result is q~*4096, undone by the exp's scale.
    F8D = mybir.dt.np(FP8)
    shared = {
        "wqp": (wqp * 256.0).astype(F8D), "wkp": (wkp / 16.0).astype(BFD),
        "wv": (np.asarray(Wv, np.float32) / 16.0).astype(BFD),
        "wo": wo_f.astype(BFD),
        "bqpc": bqpc, "bobc": bobc,
        "sel12": sel12.astype(BFD),
    }
    in_maps = [{"xt": np.ascontiguousarray(xf[c * R:(c + 1) * R].T), **shared}
               for c in range(NCORES)]
    res = run_bass_kernel_spmd(nc, in_maps, core_ids=list(range(NCORES)),
                               trace=TRACE)
    if TRACE:
        global LAST_EXEC_NS
        LAST_EXEC_NS = res.exec_time_ns
    outs = [res.results[c]["out"] for c in range(NCORES)]
    return np.concatenate(outs, axis=0).astype(np.float32).reshape(B, S, H)


# revision 26
# speedup vs baseline: 1.2078x; 1.2078x over previous
"""Low-rank (random-feature) attention kernel for Trainium2, 8 NeuronCores — v3.

Sharding: flatten hidden_states to [B*S, H] = [32768, 768] rows; core c owns
4096 contiguous rows (= batch c//2, sequence half c%2).  The per-(batch, head)
kv summary is reduced with two pipelined pairwise AllReduces (~200 KB each).

v3 structural changes vs v2 (425 us):
- Phase 1 row-tile loop reordered (all kp matmuls, then emit_kv(rt-1), then
  all v matmuls) with kp_ps double-buffered and the kv accumulators single-
  buffered: PSUM = 4+2+2 = 8 banks, and the per-tile exp/max drain now has a
  full GEMM of slack -> removes the 1.7 us PE stall per row tile (~55 us).
- bkp is folded into the q-side exp bias on the host (qp' = exp(x Wqp + bqp
  + bkp) scales numerator and denominator identically), and bv rides bo as
  bo' = bo + bv @ Wo (exact up to an O(eps/n) ~ 1e-6 term).  The whole
  post-collective kv fixup chain (ebkp/bvbc scalar_tensor_tensor ops) is
  gone; after the AllReduce only dtype-converting block-diag copies remain,
  so the normalizer matmuls no longer stall ~19 us on DVE.
- bd_kv / ks_bd memsets hoisted to the DMA load phase.
- xT row-tile loads split across the sync and vector DGE queues (halves the
  time to first real matmul).
- bf16 everywhere off-chip (including the output, cast back on host).
"""

import sys

sys.path.insert(0, "/opt/trn_rl_repo")

import contextlib

import ml_dtypes
import numpy as np

import concourse.bass as bass
import concourse.tile as tile
from concourse import mybir
from concourse.bass_utils import run_bass_kernel_spmd

BF16 = mybir.dt.bfloat16
F32 = mybir.dt.float32
FP8 = mybir.dt.float8e4
AF = mybir.ActivationFunctionType
ALU = mybir.AluOpType
AX = mybir.AxisListType

B, S, H = 4, 8192, 768
NH, HD, M = 12, 64, 64
EPS = 1e-6
NCORES = 8
R = (B * S) // NCORES          # rows per core = 4096
NPAIR = NH // 2                # head pairs = 6
KT = H // 128                  # hidden k-tiles = 6
RT = 128                       # row tile
NRT = R // RT                  # 32 row tiles
CHUNK = 512
NCH = R // CHUNK               # 8 chunks
WARM_MM = 52                   # PE warm-up matmuls during the load phase


def _split_multi_waits(nc):
    """This container's walrus only accepts one semaphore wait per
    instruction; hoist extra waits onto same-engine NoOps placed before."""
    c = 0
    for f in nc.m.functions:
        for bb in f.blocks:
            new_insts = []
            for ins in bb.instructions:
                si = ins.sync_info
                if si is not None and si.on_wait and len(si.on_wait) > 1:
                    waits = list(si.on_wait)
                    for w in waits[:-1]:
                        c += 1
                        new_insts.append(mybir.InstNoOp(
                            name=f"I-waitsplit-{c}", engine=ins.engine,
                            sync_info=mybir.SyncInfo(on_wait=[w], on_update=[])))
                    ins.sync_info = mybir.SyncInfo(
                        on_wait=[waits[-1]], on_update=list(si.on_update))
                new_insts.append(ins)
            bb.instructions[:] = new_insts
    return c


def _bc_free(ap, n):
    """Broadcast an AP along a new innermost (stride-0) free axis of size n."""
    return bass.AP(tensor=ap.tensor, offset=ap.offset, ap=list(ap.ap) + [[0, n]])


def _build(nc):
    xt = nc.dram_tensor("xt", [H, R], BF16, kind="ExternalInput")
    wkp = nc.dram_tensor("wkp", [H, H], BF16, kind="ExternalInput")
    wv = nc.dram_tensor("wv", [H, H], BF16, kind="ExternalInput")
    wqp = nc.dram_tensor("wqp", [H, H], FP8, kind="ExternalInput")
    wo = nc.dram_tensor("wo", [H, H], BF16, kind="ExternalInput")
    bqpc_d = nc.dram_tensor("bqpc", [128, KT], F32, kind="ExternalInput")
    bobc_d = nc.dram_tensor("bobc", [128, H], F32, kind="ExternalInput")
    sel12_d = nc.dram_tensor("sel12", [NH, NPAIR, 128], BF16, kind="ExternalInput")
    out = nc.dram_tensor("out", [R, H], BF16, kind="ExternalOutput")

    with tile.TileContext(nc) as tc, contextlib.ExitStack() as ctx:
        persist = ctx.enter_context(tc.tile_pool(name="persist", bufs=1))
        dram = ctx.enter_context(tc.tile_pool(name="dram", bufs=1, space="DRAM"))

        # ---- PE warm-up fodder: available immediately (no DMA dependency) ----
        junk = persist.tile([128, 512], BF16, tag="junk", name="junk")
        nc.vector.memset(junk[:], 1.0)

        # ---- constants (gpsimd SWDGE queue, small) ----
        bqpc = persist.tile([128, KT], F32, tag="bqpc", name="bqpc")
        nc.gpsimd.dma_start(out=bqpc[:], in_=bqpc_d.ap())
        bobc = persist.tile([128, H], F32, tag="bobc", name="bobc")
        nc.gpsimd.dma_start(out=bobc[:], in_=bobc_d.ap())
        sel12 = persist.tile([NH, NPAIR, 128], BF16, tag="sel12", name="sel12")
        nc.gpsimd.dma_start(out=sel12[:], in_=sel12_d.ap())

        # ---- weights + xT interleaved across the sync and scalar HWDGE rings.
        # Per-ring order is first-needed-first: wkp halves, then x chunk 0
        # (first 4 row tiles), then wv halves (one ~3 us stall at v(0) beats
        # pushing kp(0) back), then the rest ahead of phase-1 consumption.
        wkp_sb = persist.tile([128, KT, H], BF16, tag="wkp", name="wkp_sb")
        wv_sb = persist.tile([128, KT, H], BF16, tag="wv", name="wv_sb")
        wqp_sb = persist.tile([128, KT, H], FP8, tag="wqp", name="wqp_sb")
        wo_sb = persist.tile([128, KT, H], BF16, tag="wo", name="wo_sb")
        xt_sb = persist.tile([128, KT, R], BF16, tag="xt", name="xt_sb")

        def load_w(wsb, wdram, half, eng):
            ks = slice(0, 3) if half == 0 else slice(3, KT)
            eng.dma_start(out=wsb[:, ks, :],
                          in_=wdram.ap().rearrange("(k p) n -> p k n", p=128)[:, ks, :])

        # kp(0) is gated by wkp + the first x strip, so they get a full ring
        # each (2KB+ lines; the 512-col strips of v4 ran at half DMA rate and
        # starved tiles 4-7 for ~17 us).  wv lands per-kt just ahead of v(0);
        # later strips and wqp/wo trail well ahead of their consumers.
        def load_x(c0, c1, eng):
            for kt in range(KT):
                eng.dma_start(out=xt_sb[:, kt, c0:c1],
                              in_=xt[kt * 128:(kt + 1) * 128, c0:c1])

        def load_wv(kt, eng):
            eng.dma_start(out=wv_sb[:, kt, :],
                          in_=wv.ap().rearrange("(k p) n -> p k n", p=128)[:, kt, :])

        # ring A: the first x halves at full 4KB-line efficiency (they gate
        # kp(0) at ~25 us, covered by warm-up); ring B: weights in need order,
        # then the second x halves (needed from tile 16 at ~100 us).
        load_x(0, 2048, nc.sync)
        load_w(wkp_sb, wkp, 0, nc.scalar); load_w(wkp_sb, wkp, 1, nc.scalar)
        for kt in range(KT):
            load_wv(kt, nc.scalar)
        load_x(2048, 4096, nc.scalar)
        load_w(wqp_sb, wqp, 0, nc.sync); load_w(wqp_sb, wqp, 1, nc.sync)
        load_w(wo_sb, wo, 0, nc.sync); load_w(wo_sb, wo, 1, nc.sync)

        # ---- persistent result tiles ----
        qpT = [persist.tile([128, R], BF16, tag=f"qpT{p}", name=f"qpT{p}")
               for p in range(NPAIR)]
        kv_acc = [persist.tile([128, NPAIR, 130], F32, tag=f"kvacc{h}",
                               name=f"kv_acc{h}") for h in range(2)]
        kv_fix = persist.tile([128, NPAIR, 130], F32, tag="kvfix", name="kv_fix")
        bd_kv = persist.tile([128, NPAIR, 128], BF16, tag="bdkv", name="bd_kv")
        ks_bd = persist.tile([128, NPAIR, NH], BF16, tag="ksbd", name="ks_bd")
        r_cat = persist.tile([NH, R], BF16, tag="rcat", name="r_cat")
        # block-diag scaffolding zeroed while DVE is otherwise idle
        nc.vector.memset(bd_kv[:], 0.0)
        nc.vector.memset(ks_bd[:], 0.0)

        cc_in = [dram.tile([128, NPAIR * 130], F32, name=f"cc_in{h}") for h in range(2)]
        cc_out = [dram.tile([128, NPAIR * 130], F32, name=f"cc_out{h}") for h in range(2)]

        x8t = {}

        # ================= Phase 1: k/v pass + kv accumulation =================
        with contextlib.ExitStack() as kctx:
            kvsb = kctx.enter_context(tc.tile_pool(name="kvsb", bufs=2))
            kvps = kctx.enter_context(tc.tile_pool(name="kvps", bufs=1, space="PSUM"))

            # warm-up matmuls share the kp_ps PSUM slot (bufs=2)
            warm_ps = kvps.tile([128, H], F32, tag="kp_ps", bufs=2, name="warm_ps")
            for i in range(WARM_MM):
                nc.tensor.matmul(warm_ps[:, 0:512], junk[:, 0:128], junk[:, :],
                                 start=True, stop=True)

            kv_ps = [None, None]   # [kv_a, kv_b] of current half
            kp_sbs = {}
            v_sbs = {}

            def conv_x8(ch):
                # bf16 -> fp8 copy of an x chunk for the phase-2 DoubleRow
                # GEMM; chunks 0/1 convert during phase 1 so the first qp
                # matmul is not gated on DVE at the phase boundary.
                t = persist.tile([128, KT, CHUNK], FP8, tag="x8", bufs=2,
                                 name=f"xt8_{ch}")
                nc.vector.tensor_copy(
                    out=t[:], in_=xt_sb[:, :, ch * CHUNK:(ch + 1) * CHUNK])
                x8t[ch] = t

            def emit_kv(rt):
                hh, rl = rt // (NRT // 2), rt % (NRT // 2)
                if rl == 0:
                    kv_ps[0] = kvps.tile([128, 3, 130], F32, tag="kv_a",
                                         bufs=1, name=f"kv_a{hh}")
                    kv_ps[1] = kvps.tile([128, 3, 130], F32, tag="kv_b",
                                         bufs=1, name=f"kv_b{hh}")
                kp_sb, v_sb = kp_sbs.pop(rt), v_sbs.pop(rt)
                for p in range(NPAIR):
                    # start=True clears has_written for the WHOLE bank, so only
                    # the first region of each bank may issue it; the other two
                    # pairs overwrite-on-clear at rl==0 via has_written=0.
                    nc.tensor.matmul(
                        kv_ps[p // 3][:, p % 3, :],
                        kp_sb[:, p * 128:(p + 1) * 128],
                        v_sb[:, 2 * p:2 * p + 2, :],
                        start=(rl == 0 and p % 3 == 0),
                        stop=(rl == NRT // 2 - 1),
                        skip_group_check=True)

            def drain_kv(hh):
                nc.vector.tensor_copy(out=kv_acc[hh][:, 0:3, :], in_=kv_ps[0][:])
                nc.vector.tensor_copy(out=kv_acc[hh][:, 3:6, :], in_=kv_ps[1][:])
                nc.sync.dma_start(out=cc_in[hh][:],
                                  in_=kv_acc[hh].rearrange("p a b -> p (a b)"))
                nc.gpsimd.collective_compute(
                    "AllReduce", ALU.add,
                    replica_groups=[[0, 1], [2, 3], [4, 5], [6, 7]],
                    ins=[cc_in[hh].opt()], outs=[cc_out[hh].opt()])

            for rt in range(NRT):
                rs = slice(rt * RT, (rt + 1) * RT)
                kp_ps = kvps.tile([128, H], F32, tag="kp_ps", bufs=2,
                                  name=f"kp_ps{rt}")
                v_ps = kvps.tile([128, H], F32, tag="v_ps", name=f"v_ps{rt}")
                for kt in range(KT):
                    xblk = xt_sb[:, kt, rs]
                    st, sp = (kt == 0), (kt == KT - 1)
                    nc.tensor.matmul(kp_ps[:, 0:512], xblk, wkp_sb[:, kt, 0:512],
                                     start=st, stop=sp)
                    nc.tensor.matmul(kp_ps[:, 512:768], xblk, wkp_sb[:, kt, 512:768],
                                     start=st, stop=sp)
                # kv outer products for the previous row tile sit between the
                # kp and v GEMMs of this one, so the previous tile's exp/scale
                # chain and this tile's kp drain both have a GEMM of slack
                if rt > 0:
                    emit_kv(rt - 1)
                    if rt == NRT // 2:
                        drain_kv(0)
                if rt == 20:
                    conv_x8(0)
                elif rt == 24:
                    conv_x8(1)
                for kt in range(KT):
                    xblk = xt_sb[:, kt, rs]
                    st, sp = (kt == 0), (kt == KT - 1)
                    nc.tensor.matmul(v_ps[:, 0:512], xblk, wv_sb[:, kt, 0:512],
                                     start=st, stop=sp)
                    nc.tensor.matmul(v_ps[:, 512:768], xblk, wv_sb[:, kt, 512:768],
                                     start=st, stop=sp)
                # postprocess this row tile
                nmx = kvsb.tile([128, NH], F32, tag="nmx", name=f"nmx{rt}")
                nc.vector.tensor_reduce(
                    out=nmx[:], in_=kp_ps.rearrange("p (h m) -> p h m", m=M),
                    op=ALU.max, axis=AX.X, negate=True)
                emax = kvsb.tile([128, NH], F32, tag="emax", name=f"emax{rt}")
                nc.scalar.activation(out=emax[:], in_=nmx[:], func=AF.Exp)
                kp_sb = kvsb.tile([128, H], BF16, tag="kp_sb", name=f"kp_sb{rt}")
                nc.scalar.activation(out=kp_sb[:], in_=kp_ps[:], func=AF.Exp)
                v_sb = kvsb.tile([128, NH, 65], BF16, tag="v_sb", name=f"v_sb{rt}")
                nc.vector.tensor_tensor(
                    out=v_sb[:, :, 0:64],
                    in0=v_ps.rearrange("p (h d) -> p h d", d=HD),
                    in1=_bc_free(emax[:], HD), op=ALU.mult)
                nc.vector.tensor_copy(out=v_sb[:, :, 64:65], in_=emax[:])
                kp_sbs[rt], v_sbs[rt] = kp_sb, v_sb
            emit_kv(NRT - 1)
            drain_kv(1)

        # ---- collective readback + block-diag assembly (copies only) ----
        kv_r = [persist.tile([128, NPAIR, 130], F32, tag=f"kvr{h}", name=f"kv_r{h}")
                for h in range(2)]
        for hh in range(2):
            nc.sync.dma_start(out=kv_r[hh][:],
                              in_=cc_out[hh].rearrange("p (a b) -> p a b", b=130))
        nc.vector.tensor_add(out=kv_fix[:], in0=kv_r[0][:], in1=kv_r[1][:])
        for p in range(NPAIR):
            nc.vector.tensor_copy(out=bd_kv[0:64, p, 0:64], in_=kv_fix[0:64, p, 0:64])
            nc.vector.tensor_copy(out=bd_kv[64:128, p, 64:128],
                                  in_=kv_fix[64:128, p, 65:129])
            nc.vector.tensor_copy(out=ks_bd[0:64, p, 2 * p:2 * p + 1],
                                  in_=kv_fix[0:64, p, 64:65])
            nc.vector.tensor_copy(out=ks_bd[64:128, p, 2 * p + 1:2 * p + 2],
                                  in_=kv_fix[64:128, p, 129:130])

        # ================= Phase 2: q pass, then normalizer =================
        with contextlib.ExitStack() as qctx:
            qsb = qctx.enter_context(tc.tile_pool(name="qsb", bufs=2))
            qps = qctx.enter_context(tc.tile_pool(name="qps", bufs=1, space="PSUM"))

            def norm_ch(ch):
                # 1/n computed as exp(-ln n) on the scalar engine: n is O(1e3)
                # and positive so eps and the Ln LUT error are negligible,
                # while the DVE reciprocal (3.3 us per chunk, serial) was
                # gating phase 3's PSUM bank reuse by ~12 us.
                cs = slice(ch * CHUNK, (ch + 1) * CHUNK)
                n_ps = qps.tile([NH, CHUNK], F32, tag="np", bufs=3, name=f"n_ps{ch}")
                for p in range(NPAIR):
                    nc.tensor.matmul(n_ps[:], ks_bd[:, p, :], qpT[p][:, cs],
                                     start=(p == 0), stop=(p == NPAIR - 1))
                ln_sb = qsb.tile([NH, CHUNK], F32, tag="lnn", bufs=2,
                                 name=f"ln_n{ch}")
                nc.scalar.activation(out=ln_sb[:], in_=n_ps[:], func=AF.Ln)
                nc.scalar.activation(out=r_cat[:, cs], in_=ln_sb[:], func=AF.Exp,
                                     scale=-1.0)

            for ch in range(NCH):
                cs = slice(ch * CHUNK, (ch + 1) * CHUNK)
                # q-side runs in fp8 DoubleRow (x16 and wqp x256 land both
                # operands in e4m3 range; the 1/4096 rides the exp's scale).
                # The k-side must stay bf16 (fp8 kp alone costs 2e-2 rel err),
                # the q-side alone costs ~9e-3 against the 2e-2 budget.
                xt8 = x8t[ch]
                for ct in range(KT):
                    qp_ps = qps.tile([128, CHUNK], F32, tag="qp", bufs=3,
                                     name=f"qp_ps{ch}_{ct}")
                    for kt in range(0, KT, 2):
                        nc.tensor.matmul(
                            qp_ps[:],
                            wqp_sb[:, kt:kt + 2, ct * 128:(ct + 1) * 128],
                            xt8[:, kt:kt + 2, :],
                            start=(kt == 0), stop=(kt == KT - 2),
                            perf_mode=mybir.MatmulPerfMode.DoubleRow)
                    nc.scalar.activation(out=qpT[ct][:, cs], in_=qp_ps[:],
                                         func=AF.Exp, scale=1.0 / 4096.0,
                                         bias=bqpc[:, ct:ct + 1])
                if ch + 2 < NCH:
                    conv_x8(ch + 2)
                # normalizer chunks ride along once ks_bd (collective +
                # readback, ~215 us) is ready, paced across the remaining
                # qp chunks
                if ch >= 5:
                    norm_ch(ch - 5)
            for ch in range(NCH - 5, NCH):
                norm_ch(ch)

        # ================= Phase 3: ctx + output projection =================
        with contextlib.ExitStack() as cctx:
            csb = cctx.enter_context(tc.tile_pool(name="csb", bufs=2))
            cps = cctx.enter_context(tc.tile_pool(name="cps", bufs=1, space="PSUM"))
            for ch in range(NCH):
                cs = slice(ch * CHUNK, (ch + 1) * CHUNK)
                ctx_ch = csb.tile([128, NPAIR, CHUNK], BF16, tag="ctx",
                                  name=f"ctx{ch}")
                for p in range(NPAIR):
                    rb_ps = cps.tile([128, CHUNK], F32, tag="rb", bufs=2,
                                     name=f"rb_ps{ch}_{p}")
                    nc.tensor.matmul(rb_ps[:], sel12[:, p, :], r_cat[:, cs],
                                     start=True, stop=True)
                    rb_sb = csb.tile([128, CHUNK], F32, tag="rbsb", bufs=2,
                                     name=f"rb_sb{ch}_{p}")
                    nc.vector.tensor_copy(out=rb_sb[:], in_=rb_ps[:])
                    a_ps = cps.tile([128, CHUNK], F32, tag="a", bufs=2,
                                    name=f"a_ps{ch}_{p}")
                    nc.tensor.matmul(a_ps[:], bd_kv[:, p, :], qpT[p][:, cs],
                                     start=True, stop=True)
                    nc.vector.tensor_tensor(out=ctx_ch[:, p, :], in0=a_ps[:],
                                            in1=rb_sb[:], op=ALU.mult)
                for r4 in range(CHUNK // RT):
                    rt = ch * (CHUNK // RT) + r4
                    rs = slice(r4 * RT, (r4 + 1) * RT)
                    o_ps = cps.tile([128, H], F32, tag="o", bufs=2,
                                    name=f"o_ps{rt}")
                    for p in range(NPAIR):
                        st, sp = (p == 0), (p == NPAIR - 1)
                        nc.tensor.matmul(o_ps[:, 0:512], ctx_ch[:, p, rs],
                                         wo_sb[:, p, 0:512], start=st, stop=sp)
                        nc.tensor.matmul(o_ps[:, 512:768], ctx_ch[:, p, rs],
                                         wo_sb[:, p, 512:768], start=st, stop=sp)
                    o_sb = csb.tile([128, H], BF16, tag="osb", bufs=3,
                                    name=f"o_sb{rt}")
                    nc.vector.tensor_tensor(out=o_sb[:], in0=o_ps[:], in1=bobc[:],
                                            op=ALU.add)
                    nc.sync.dma_start(out=out[rt * RT:(rt + 1) * RT, :], in_=o_sb[:])

    _split_multi_waits(nc)
    return nc


_CACHE = {}
TRACE = False          # set by test harness to capture an NTFF profile
LAST_EXEC_NS = None    # filled on a TRACE run


def _get_nc():
    if "nc" not in _CACHE:
        nc = bass.Bass("TRN2", target_bir_lowering=False, debug=False,
                       num_devices=NCORES)
        _CACHE["nc"] = _build(nc)
    return _CACHE["nc"]


def kernel(hidden_states, Wq, bq, Wk, bk, Wv, bv, Wo, bo, projection_matrix):
    nc = _get_nc()
    BFD = ml_dtypes.bfloat16
    xf = np.asarray(hidden_states, dtype=np.float32).reshape(B * S, H)
    xf = (xf * np.float32(16.0)).astype(BFD)
    pm = np.asarray(projection_matrix, dtype=np.float32)
    wq_f = np.asarray(Wq, dtype=np.float32)
    wk_f = np.asarray(Wk, dtype=np.float32)
    wo_f = np.asarray(Wo, dtype=np.float32)
    bq_f = np.asarray(bq, dtype=np.float32)
    bk_f = np.asarray(bk, dtype=np.float32)
    bv_f = np.asarray(bv, dtype=np.float32)
    bo_f = np.asarray(bo, dtype=np.float32)
    # fold the feature projection into the q and k weights (exact in fp32)
    wqp = np.zeros((H, H), np.float32)
    wkp = np.zeros((H, H), np.float32)
    bqp = np.zeros((H,), np.float32)
    bkp = np.zeros((H,), np.float32)
    for h in range(NH):
        cols = slice(h * HD, (h + 1) * HD)
        wqp[:, cols] = wq_f[:, cols] @ pm[h]
        wkp[:, cols] = wk_f[:, cols] @ pm[h]
        bqp[cols] = bq_f[cols] @ pm[h]
        bkp[cols] = bk_f[cols] @ pm[h]
    # k-side projected bias rides the q-side exp (it scales the ctx numerator
    # and denominator identically); bv rides bo through Wo.
    bqp = bqp + bkp
    bo_f = bo_f + bv_f @ wo_f
    bqpc = np.ascontiguousarray(bqp.reshape(KT, 128).T)            # [128, KT]
    bobc = np.ascontiguousarray(np.broadcast_to(bo_f, (128, H)).copy())
    sel12 = np.zeros((NH, NPAIR, 128), np.float32)
    for p in range(NPAIR):
        sel12[2 * p, p, 0:64] = 1.0
        sel12[2 * p + 1, p, 64:128] = 1.0
    # power-of-2 rescales (exact in bf16): x*16 lifts the fp8 copy of x out
    # of e4m3 subnormals, wkp/wv absorb the 1/16, wqp*256 centers the fp8
    # weights; the q GEMM result is q~*4096, undone by the exp's scale.
    F8D = mybir.dt.np(FP8)
    shared = {
        "wqp": (wqp * 256.0).astype(F8D), "wkp": (wkp / 16.0).astype(BFD),
        "wv": (np.asarray(Wv, np.float32) / 16.0).astype(BFD),
        "wo": wo_f.astype(BFD),
        "bqpc": bqpc, "bobc": bobc,
        "sel12": sel12.astype(BFD),
    }
    in_maps = [{"xt": np.ascontiguousarray(xf[c * R:(c + 1) * R].T), **shared}
               for c in range(NCORES)]
    res = run_bass_kernel_spmd(nc, in_maps, core_ids=list(range(NCORES)),
                               trace=TRACE)
    if TRACE:
        global LAST_EXEC_NS
        LAST_EXEC_NS = res.exec_time_ns
    outs = [res.results[c]["out"] for c in range(NCORES)]
    return np.concatenate(outs, axis=0).astype(np.float32).reshape(B, S, H)


# revision 27
# speedup vs baseline: 1.2228x; 1.0125x over previous
"""Low-rank (random-feature) attention kernel for Trainium2, 8 NeuronCores — v3.

Sharding: flatten hidden_states to [B*S, H] = [32768, 768] rows; core c owns
4096 contiguous rows (= batch c//2, sequence half c%2).  The per-(batch, head)
kv summary is reduced with two pipelined pairwise AllReduces (~200 KB each).

v3 structural changes vs v2 (425 us):
- Phase 1 row-tile loop reordered (all kp matmuls, then emit_kv(rt-1), then
  all v matmuls) with kp_ps double-buffered and the kv accumulators single-
  buffered: PSUM = 4+2+2 = 8 banks, and the per-tile exp/max drain now has a
  full GEMM of slack -> removes the 1.7 us PE stall per row tile (~55 us).
- bkp is folded into the q-side exp bias on the host (qp' = exp(x Wqp + bqp
  + bkp) scales numerator and denominator identically), and bv rides bo as
  bo' = bo + bv @ Wo (exact up to an O(eps/n) ~ 1e-6 term).  The whole
  post-collective kv fixup chain (ebkp/bvbc scalar_tensor_tensor ops) is
  gone; after the AllReduce only dtype-converting block-diag copies remain,
  so the normalizer matmuls no longer stall ~19 us on DVE.
- bd_kv / ks_bd memsets hoisted to the DMA load phase.
- xT row-tile loads split across the sync and vector DGE queues (halves the
  time to first real matmul).
- bf16 everywhere off-chip (including the output, cast back on host).
"""

import sys

sys.path.insert(0, "/opt/trn_rl_repo")

import contextlib

import ml_dtypes
import numpy as np

import concourse.bass as bass
import concourse.tile as tile
from concourse import mybir
from concourse.bass_utils import run_bass_kernel_spmd

BF16 = mybir.dt.bfloat16
F32 = mybir.dt.float32
FP8 = mybir.dt.float8e4
AF = mybir.ActivationFunctionType
ALU = mybir.AluOpType
AX = mybir.AxisListType

B, S, H = 4, 8192, 768
NH, HD, M = 12, 64, 64
EPS = 1e-6
NCORES = 8
R = (B * S) // NCORES          # rows per core = 4096
NPAIR = NH // 2                # head pairs = 6
KT = H // 128                  # hidden k-tiles = 6
RT = 128                       # row tile
NRT = R // RT                  # 32 row tiles
CHUNK = 512
NCH = R // CHUNK               # 8 chunks
WARM_MM = 36                   # PE warm-up matmuls during the load phase


def _split_multi_waits(nc):
    """This container's walrus only accepts one semaphore wait per
    instruction; hoist extra waits onto same-engine NoOps placed before."""
    c = 0
    for f in nc.m.functions:
        for bb in f.blocks:
            new_insts = []
            for ins in bb.instructions:
                si = ins.sync_info
                if si is not None and si.on_wait and len(si.on_wait) > 1:
                    waits = list(si.on_wait)
                    for w in waits[:-1]:
                        c += 1
                        new_insts.append(mybir.InstNoOp(
                            name=f"I-waitsplit-{c}", engine=ins.engine,
                            sync_info=mybir.SyncInfo(on_wait=[w], on_update=[])))
                    ins.sync_info = mybir.SyncInfo(
                        on_wait=[waits[-1]], on_update=list(si.on_update))
                new_insts.append(ins)
            bb.instructions[:] = new_insts
    return c


def _bc_free(ap, n):
    """Broadcast an AP along a new innermost (stride-0) free axis of size n."""
    return bass.AP(tensor=ap.tensor, offset=ap.offset, ap=list(ap.ap) + [[0, n]])


def _build(nc):
    xt = nc.dram_tensor("xt", [H, R], BF16, kind="ExternalInput")
    wkp = nc.dram_tensor("wkp", [H, H], BF16, kind="ExternalInput")
    wv = nc.dram_tensor("wv", [H, H], BF16, kind="ExternalInput")
    wqp = nc.dram_tensor("wqp", [H, H], FP8, kind="ExternalInput")
    wo = nc.dram_tensor("wo", [H, H], BF16, kind="ExternalInput")
    bqpc_d = nc.dram_tensor("bqpc", [128, KT], F32, kind="ExternalInput")
    bobc_d = nc.dram_tensor("bobc", [128, H], F32, kind="ExternalInput")
    sel12_d = nc.dram_tensor("sel12", [NH, NPAIR, 128], BF16, kind="ExternalInput")
    out = nc.dram_tensor("out", [R, H], BF16, kind="ExternalOutput")

    with tile.TileContext(nc) as tc, contextlib.ExitStack() as ctx:
        persist = ctx.enter_context(tc.tile_pool(name="persist", bufs=1))
        dram = ctx.enter_context(tc.tile_pool(name="dram", bufs=1, space="DRAM"))

        # ---- PE warm-up fodder: available immediately (no DMA dependency) ----
        junk = persist.tile([128, 512], BF16, tag="junk", name="junk")
        nc.vector.memset(junk[:], 1.0)

        # ---- constants (gpsimd SWDGE queue, small) ----
        bqpc = persist.tile([128, KT], F32, tag="bqpc", name="bqpc")
        nc.gpsimd.dma_start(out=bqpc[:], in_=bqpc_d.ap())
        bobc = persist.tile([128, H], F32, tag="bobc", name="bobc")
        nc.gpsimd.dma_start(out=bobc[:], in_=bobc_d.ap())
        sel12 = persist.tile([NH, NPAIR, 128], BF16, tag="sel12", name="sel12")
        nc.gpsimd.dma_start(out=sel12[:], in_=sel12_d.ap())

        # ---- weights + xT interleaved across the sync and scalar HWDGE rings.
        # Per-ring order is first-needed-first: wkp halves, then x chunk 0
        # (first 4 row tiles), then wv halves (one ~3 us stall at v(0) beats
        # pushing kp(0) back), then the rest ahead of phase-1 consumption.
        wkp_sb = persist.tile([128, KT, H], BF16, tag="wkp", name="wkp_sb")
        wv_sb = persist.tile([128, KT, H], BF16, tag="wv", name="wv_sb")
        wqp_sb = persist.tile([128, KT, H], FP8, tag="wqp", name="wqp_sb")
        wo_sb = persist.tile([128, KT, H], BF16, tag="wo", name="wo_sb")
        xt_sb = persist.tile([128, KT, R], BF16, tag="xt", name="xt_sb")

        def load_w(wsb, wdram, half, eng):
            ks = slice(0, 3) if half == 0 else slice(3, KT)
            eng.dma_start(out=wsb[:, ks, :],
                          in_=wdram.ap().rearrange("(k p) n -> p k n", p=128)[:, ks, :])

        # kp(0) is gated by wkp + the first x strip, so they get a full ring
        # each (2KB+ lines; the 512-col strips of v4 ran at half DMA rate and
        # starved tiles 4-7 for ~17 us).  wv lands per-kt just ahead of v(0);
        # later strips and wqp/wo trail well ahead of their consumers.
        def load_x(c0, c1, eng):
            for kt in range(KT):
                eng.dma_start(out=xt_sb[:, kt, c0:c1],
                              in_=xt[kt * 128:(kt + 1) * 128, c0:c1])

        def load_wv(kt, eng):
            eng.dma_start(out=wv_sb[:, kt, :],
                          in_=wv.ap().rearrange("(k p) n -> p k n", p=128)[:, kt, :])

        # kp(0) needs wkp + the first x halves (4.25 MB): split both across
        # the two rings so it lands at ~19 us instead of ~25; wv follows (one
        # ~3 us stall at v(0) beats delaying kp(0)); second x halves arrive
        # ~37 us, well before tile 16 consumes them at ~95 us.
        def load_x1(kt, c0, c1, eng):
            eng.dma_start(out=xt_sb[:, kt, c0:c1],
                          in_=xt[kt * 128:(kt + 1) * 128, c0:c1])

        for half, eng in ((0, nc.sync), (1, nc.scalar)):
            load_w(wkp_sb, wkp, half, eng)
            for kt in (0, 2, 4) if half == 0 else (1, 3, 5):
                load_x1(kt, 0, 2048, eng)
            load_w(wv_sb, wv, half, eng)
            for kt in (0, 2, 4) if half == 0 else (1, 3, 5):
                load_x1(kt, 2048, 4096, eng)
            load_w(wqp_sb, wqp, half, eng)
            load_w(wo_sb, wo, half, eng)

        # ---- persistent result tiles ----
        qpT = [persist.tile([128, R], BF16, tag=f"qpT{p}", name=f"qpT{p}")
               for p in range(NPAIR)]
        kv_acc = [persist.tile([128, NPAIR, 130], F32, tag=f"kvacc{h}",
                               name=f"kv_acc{h}") for h in range(2)]
        kv_fix = persist.tile([128, NPAIR, 130], F32, tag="kvfix", name="kv_fix")
        bd_kv = persist.tile([128, NPAIR, 128], BF16, tag="bdkv", name="bd_kv")
        ks_bd = persist.tile([128, NPAIR, NH], BF16, tag="ksbd", name="ks_bd")
        r_cat = persist.tile([NH, R], BF16, tag="rcat", name="r_cat")
        # block-diag scaffolding zeroed while DVE is otherwise idle
        nc.vector.memset(bd_kv[:], 0.0)
        nc.vector.memset(ks_bd[:], 0.0)

        cc_in = [dram.tile([128, NPAIR * 130], F32, name=f"cc_in{h}") for h in range(2)]
        cc_out = [dram.tile([128, NPAIR * 130], F32, name=f"cc_out{h}") for h in range(2)]

        x8t = {}

        # ================= Phase 1: k/v pass + kv accumulation =================
        with contextlib.ExitStack() as kctx:
            kvsb = kctx.enter_context(tc.tile_pool(name="kvsb", bufs=2))
            kvps = kctx.enter_context(tc.tile_pool(name="kvps", bufs=1, space="PSUM"))

            # warm-up matmuls share the kp_ps PSUM slot (bufs=2)
            warm_ps = kvps.tile([128, H], F32, tag="kp_ps", bufs=2, name="warm_ps")
            for i in range(WARM_MM):
                nc.tensor.matmul(warm_ps[:, 0:512], junk[:, 0:128], junk[:, :],
                                 start=True, stop=True)

            kv_ps = [None, None]   # [kv_a, kv_b] of current half
            kp_sbs = {}
            v_sbs = {}

            def conv_x8(ch):
                # bf16 -> fp8 copy of an x chunk for the phase-2 DoubleRow
                # GEMM; chunks 0/1 convert during phase 1 so the first qp
                # matmul is not gated on DVE at the phase boundary.
                t = persist.tile([128, KT, CHUNK], FP8, tag="x8", bufs=2,
                                 name=f"xt8_{ch}")
                nc.vector.tensor_copy(
                    out=t[:], in_=xt_sb[:, :, ch * CHUNK:(ch + 1) * CHUNK])
                x8t[ch] = t

            def emit_kv(rt):
                hh, rl = rt // (NRT // 2), rt % (NRT // 2)
                if rl == 0:
                    kv_ps[0] = kvps.tile([128, 3, 130], F32, tag="kv_a",
                                         bufs=1, name=f"kv_a{hh}")
                    kv_ps[1] = kvps.tile([128, 3, 130], F32, tag="kv_b",
                                         bufs=1, name=f"kv_b{hh}")
                kp_sb, v_sb = kp_sbs.pop(rt), v_sbs.pop(rt)
                for p in range(NPAIR):
                    # start=True clears has_written for the WHOLE bank, so only
                    # the first region of each bank may issue it; the other two
                    # pairs overwrite-on-clear at rl==0 via has_written=0.
                    nc.tensor.matmul(
                        kv_ps[p // 3][:, p % 3, :],
                        kp_sb[:, p * 128:(p + 1) * 128],
                        v_sb[:, 2 * p:2 * p + 2, :],
                        start=(rl == 0 and p % 3 == 0),
                        stop=(rl == NRT // 2 - 1),
                        skip_group_check=True)

            def drain_kv(hh):
                nc.vector.tensor_copy(out=kv_acc[hh][:, 0:3, :], in_=kv_ps[0][:])
                nc.vector.tensor_copy(out=kv_acc[hh][:, 3:6, :], in_=kv_ps[1][:])
                nc.sync.dma_start(out=cc_in[hh][:],
                                  in_=kv_acc[hh].rearrange("p a b -> p (a b)"))
                nc.gpsimd.collective_compute(
                    "AllReduce", ALU.add,
                    replica_groups=[[0, 1], [2, 3], [4, 5], [6, 7]],
                    ins=[cc_in[hh].opt()], outs=[cc_out[hh].opt()])

            for rt in range(NRT):
                rs = slice(rt * RT, (rt + 1) * RT)
                kp_ps = kvps.tile([128, H], F32, tag="kp_ps", bufs=2,
                                  name=f"kp_ps{rt}")
                v_ps = kvps.tile([128, H], F32, tag="v_ps", name=f"v_ps{rt}")
                for kt in range(KT):
                    xblk = xt_sb[:, kt, rs]
                    st, sp = (kt == 0), (kt == KT - 1)
                    nc.tensor.matmul(kp_ps[:, 0:512], xblk, wkp_sb[:, kt, 0:512],
                                     start=st, stop=sp)
                    nc.tensor.matmul(kp_ps[:, 512:768], xblk, wkp_sb[:, kt, 512:768],
                                     start=st, stop=sp)
                # kv outer products for the previous row tile sit between the
                # kp and v GEMMs of this one, so the previous tile's exp/scale
                # chain and this tile's kp drain both have a GEMM of slack
                if rt > 0:
                    emit_kv(rt - 1)
                    if rt == NRT // 2:
                        drain_kv(0)
                if rt == 20:
                    conv_x8(0)
                elif rt == 24:
                    conv_x8(1)
                for kt in range(KT):
                    xblk = xt_sb[:, kt, rs]
                    st, sp = (kt == 0), (kt == KT - 1)
                    nc.tensor.matmul(v_ps[:, 0:512], xblk, wv_sb[:, kt, 0:512],
                                     start=st, stop=sp)
                    nc.tensor.matmul(v_ps[:, 512:768], xblk, wv_sb[:, kt, 512:768],
                                     start=st, stop=sp)
                # postprocess this row tile
                nmx = kvsb.tile([128, NH], F32, tag="nmx", name=f"nmx{rt}")
                nc.vector.tensor_reduce(
                    out=nmx[:], in_=kp_ps.rearrange("p (h m) -> p h m", m=M),
                    op=ALU.max, axis=AX.X, negate=True)
                emax = kvsb.tile([128, NH], F32, tag="emax", name=f"emax{rt}")
                nc.scalar.activation(out=emax[:], in_=nmx[:], func=AF.Exp)
                kp_sb = kvsb.tile([128, H], BF16, tag="kp_sb", name=f"kp_sb{rt}")
                nc.scalar.activation(out=kp_sb[:], in_=kp_ps[:], func=AF.Exp)
                v_sb = kvsb.tile([128, NH, 65], BF16, tag="v_sb", name=f"v_sb{rt}")
                nc.vector.tensor_tensor(
                    out=v_sb[:, :, 0:64],
                    in0=v_ps.rearrange("p (h d) -> p h d", d=HD),
                    in1=_bc_free(emax[:], HD), op=ALU.mult)
                nc.vector.tensor_copy(out=v_sb[:, :, 64:65], in_=emax[:])
                kp_sbs[rt], v_sbs[rt] = kp_sb, v_sb
            emit_kv(NRT - 1)
            drain_kv(1)

        # ---- collective readback + block-diag assembly (copies only) ----
        kv_r = [persist.tile([128, NPAIR, 130], F32, tag=f"kvr{h}", name=f"kv_r{h}")
                for h in range(2)]
        for hh in range(2):
            nc.sync.dma_start(out=kv_r[hh][:],
                              in_=cc_out[hh].rearrange("p (a b) -> p a b", b=130))
        nc.vector.tensor_add(out=kv_fix[:], in0=kv_r[0][:], in1=kv_r[1][:])
        for p in range(NPAIR):
            nc.vector.tensor_copy(out=bd_kv[0:64, p, 0:64], in_=kv_fix[0:64, p, 0:64])
            nc.vector.tensor_copy(out=bd_kv[64:128, p, 64:128],
                                  in_=kv_fix[64:128, p, 65:129])
            nc.vector.tensor_copy(out=ks_bd[0:64, p, 2 * p:2 * p + 1],
                                  in_=kv_fix[0:64, p, 64:65])
            nc.vector.tensor_copy(out=ks_bd[64:128, p, 2 * p + 1:2 * p + 2],
                                  in_=kv_fix[64:128, p, 129:130])

        # ================= Phase 2: q pass, then normalizer =================
        with contextlib.ExitStack() as qctx:
            qsb = qctx.enter_context(tc.tile_pool(name="qsb", bufs=2))
            qps = qctx.enter_context(tc.tile_pool(name="qps", bufs=1, space="PSUM"))

            def norm_ch(ch):
                # 1/n computed as exp(-ln n) on the scalar engine: n is O(1e3)
                # and positive so eps and the Ln LUT error are negligible,
                # while the DVE reciprocal (3.3 us per chunk, serial) was
                # gating phase 3's PSUM bank reuse by ~12 us.
                cs = slice(ch * CHUNK, (ch + 1) * CHUNK)
                n_ps = qps.tile([NH, CHUNK], F32, tag="np", bufs=3, name=f"n_ps{ch}")
                for p in range(NPAIR):
                    nc.tensor.matmul(n_ps[:], ks_bd[:, p, :], qpT[p][:, cs],
                                     start=(p == 0), stop=(p == NPAIR - 1))
                ln_sb = qsb.tile([NH, CHUNK], F32, tag="lnn", bufs=2,
                                 name=f"ln_n{ch}")
                nc.scalar.activation(out=ln_sb[:], in_=n_ps[:], func=AF.Ln)
                nc.scalar.activation(out=r_cat[:, cs], in_=ln_sb[:], func=AF.Exp,
                                     scale=-1.0)

            for ch in range(NCH):
                cs = slice(ch * CHUNK, (ch + 1) * CHUNK)
                # q-side runs in fp8 DoubleRow (x16 and wqp x256 land both
                # operands in e4m3 range; the 1/4096 rides the exp's scale).
                # The k-side must stay bf16 (fp8 kp alone costs 2e-2 rel err),
                # the q-side alone costs ~9e-3 against the 2e-2 budget.
                xt8 = x8t[ch]
                for ct in range(KT):
                    qp_ps = qps.tile([128, CHUNK], F32, tag="qp", bufs=3,
                                     name=f"qp_ps{ch}_{ct}")
                    for kt in range(0, KT, 2):
                        nc.tensor.matmul(
                            qp_ps[:],
                            wqp_sb[:, kt:kt + 2, ct * 128:(ct + 1) * 128],
                            xt8[:, kt:kt + 2, :],
                            start=(kt == 0), stop=(kt == KT - 2),
                            perf_mode=mybir.MatmulPerfMode.DoubleRow)
                    nc.scalar.activation(out=qpT[ct][:, cs], in_=qp_ps[:],
                                         func=AF.Exp, scale=1.0 / 4096.0,
                                         bias=bqpc[:, ct:ct + 1])
                if ch + 2 < NCH:
                    conv_x8(ch + 2)
                # normalizer chunks ride along once ks_bd (collective +
                # readback, ~215 us) is ready, paced across the remaining
                # qp chunks
                if ch >= 5:
                    norm_ch(ch - 5)
            for ch in range(NCH - 5, NCH):
                norm_ch(ch)

        # ================= Phase 3: ctx + output projection =================
        with contextlib.ExitStack() as cctx:
            csb = cctx.enter_context(tc.tile_pool(name="csb", bufs=2))
            cps = cctx.enter_context(tc.tile_pool(name="cps", bufs=1, space="PSUM"))
            for ch in range(NCH):
                cs = slice(ch * CHUNK, (ch + 1) * CHUNK)
                ctx_ch = csb.tile([128, NPAIR, CHUNK], BF16, tag="ctx",
                                  name=f"ctx{ch}")
                for p in range(NPAIR):
                    rb_ps = cps.tile([128, CHUNK], F32, tag="rb", bufs=2,
                                     name=f"rb_ps{ch}_{p}")
                    nc.tensor.matmul(rb_ps[:], sel12[:, p, :], r_cat[:, cs],
                                     start=True, stop=True)
                    rb_sb = csb.tile([128, CHUNK], F32, tag="rbsb", bufs=2,
                                     name=f"rb_sb{ch}_{p}")
                    nc.vector.tensor_copy(out=rb_sb[:], in_=rb_ps[:])
                    a_ps = cps.tile([128, CHUNK], F32, tag="a", bufs=2,
                                    name=f"a_ps{ch}_{p}")
                    nc.tensor.matmul(a_ps[:], bd_kv[:, p, :], qpT[p][:, cs],
                                     start=True, stop=True)
                    nc.vector.tensor_tensor(out=ctx_ch[:, p, :], in0=a_ps[:],
                                            in1=rb_sb[:], op=ALU.mult)
                for r4 in range(CHUNK // RT):
                    rt = ch * (CHUNK // RT) + r4
                    rs = slice(r4 * RT, (r4 + 1) * RT)
                    o_ps = cps.tile([128, H], F32, tag="o", bufs=2,
                                    name=f"o_ps{rt}")
                    for p in range(NPAIR):
                        st, sp = (p == 0), (p == NPAIR - 1)
                        nc.tensor.matmul(o_ps[:, 0:512], ctx_ch[:, p, rs],
                                         wo_sb[:, p, 0:512], start=st, stop=sp)
                        nc.tensor.matmul(o_ps[:, 512:768], ctx_ch[:, p, rs],
                                         wo_sb[:, p, 512:768], start=st, stop=sp)
                    o_sb = csb.tile([128, H], BF16, tag="osb", bufs=3,
                                    name=f"o_sb{rt}")
                    nc.vector.tensor_tensor(out=o_sb[:], in0=o_ps[:], in1=bobc[:],
                                            op=ALU.add)
                    nc.sync.dma_start(out=out[rt * RT:(rt + 1) * RT, :], in_=o_sb[:])

    _split_multi_waits(nc)
    return nc


_CACHE = {}
TRACE = False          # set by test harness to capture an NTFF profile
LAST_EXEC_NS = None    # filled on a TRACE run


def _get_nc():
    if "nc" not in _CACHE:
        nc = bass.Bass("TRN2", target_bir_lowering=False, debug=False,
                       num_devices=NCORES)
        _CACHE["nc"] = _build(nc)
    return _CACHE["nc"]


def kernel(hidden_states, Wq, bq, Wk, bk, Wv, bv, Wo, bo, projection_matrix):
    nc = _get_nc()
    BFD = ml_dtypes.bfloat16
    xf = np.asarray(hidden_states, dtype=np.float32).reshape(B * S, H)
    xf = (xf * np.float32(16.0)).astype(BFD)
    pm = np.asarray(projection_matrix, dtype=np.float32)
    wq_f = np.asarray(Wq, dtype=np.float32)
    wk_f = np.asarray(Wk, dtype=np.float32)
    wo_f = np.asarray(Wo, dtype=np.float32)
    bq_f = np.asarray(bq, dtype=np.float32)
    bk_f = np.asarray(bk, dtype=np.float32)
    bv_f = np.asarray(bv, dtype=np.float32)
    bo_f = np.asarray(bo, dtype=np.float32)
    # fold the feature projection into the q and k weights (exact in fp32)
    wqp = np.zeros((H, H), np.float32)
    wkp = np.zeros((H, H), np.float32)
    bqp = np.zeros((H,), np.float32)
    bkp = np.zeros((H,), np.float32)
    for h in range(NH):
        cols = slice(h * HD, (h + 1) * HD)
        wqp[:, cols] = wq_f[:, cols] @ pm[h]
        wkp[:, cols] = wk_f[:, cols] @ pm[h]
        bqp[cols] = bq_f[cols] @ pm[h]
        bkp[cols] = bk_f[cols] @ pm[h]
    # k-side projected bias rides the q-side exp (it scales the ctx numerator
    # and denominator identically); bv rides bo through Wo.
    bqp = bqp + bkp
    bo_f = bo_f + bv_f @ wo_f
    bqpc = np.ascontiguousarray(bqp.reshape(KT, 128).T)            # [128, KT]
    bobc = np.ascontiguousarray(np.broadcast_to(bo_f, (128, H)).copy())
    sel12 = np.zeros((NH, NPAIR, 128), np.float32)
    for p in range(NPAIR):
        sel12[2 * p, p, 0:64] = 1.0
        sel12[2 * p + 1, p, 64:128] = 1.0
    # power-of-2 rescales (exact in bf16): x*16 lifts the fp8 copy of x out
    # of e4m3 subnormals, wkp/wv absorb the 1/16, wqp*256 centers the fp8
    # weights; the q GEMM result is q~*4096, undone by the exp's scale.
    F8D = mybir.dt.np(FP8)
    shared = {
        "wqp": (wqp * 256.0).astype(F8D), "wkp": (wkp / 16.0).astype(BFD),
        "wv": (np.asarray(Wv, np.float32) / 16.0).astype(BFD),
        "wo": wo_f.astype(BFD),
        "bqpc": bqpc, "bobc": bobc,
        "sel12": sel12.astype(BFD),
    }
    in_maps = [{"xt": np.ascontiguousarray(xf[c * R:(c + 1) * R].T), **shared}
               for c in range(NCORES)]
    res = run_bass_kernel_spmd(nc, in_maps, core_ids=list(range(NCORES)),
                               trace=TRACE)
    if TRACE:
        global LAST_EXEC_NS
        LAST_EXEC_NS = res.exec_time_ns
    outs = [res.results[c]["out"] for c in range(NCORES)]
    return np.concatenate(outs, axis=0).astype(np.float32).reshape(B, S, H)
